# revision 3
# baseline (speedup 1.0000x reference)
"""Trainium2 Bass kernel for an AttentionBlock (GroupNorm + single-head
self-attention + projection + residual), data-parallel over batch on 8
NeuronCores.

Reference computation (per batch element, S = H*W = 4096, C = 256):
    xn   = GroupNorm(x, groups=8, eps=1e-3) * gamma + beta
    q    = xn @ Wq + bq ; k = xn @ Wk + bk ; v = xn @ Wv + bv
    attn = softmax((q @ k^T) / sqrt(C))
    out  = attn @ v
    y    = xn + (out @ Wp + bp)

Layout strategy (per core, 2 batch elements, processed sequentially):
  - x is loaded in natural [s, c] tiles and PE-transposed to x_T [c, s].
  - GroupNorm stats come from free-dim reductions on x_T plus two tiny
    cross-partition matmuls with group indicator matrices.
  - Normalization is applied per-channel (partition scalars) on x_T,
    producing xn_T in fp32 (for the residual) and bf16 (matmul input).
  - q_T/k_T are produced in [c, s] layout, v in [t, c] layout, so the
    score matmul produces scores_T in [t, s] layout and attn @ v needs
    NO transposes of the attention matrix.
  - softmax skips the max-subtraction (scores are ~N(0,1); exp is safe)
    so everything stays in [t, s] layout; the denominator is computed
    with a ones-vector matmul and division is deferred to the very end,
    where it is a per-partition scalar in [s, c] layout.
  - The residual add reuses the PE transpose of xn_T back to [s, c].
"""

import os
import sys

for _p in ("/opt/trn_rl_repo", "/root/.axon_site/_ro/trn_rl_repo"):
    if os.path.isdir(_p) and _p not in sys.path:
        sys.path.append(_p)

import numpy as np
import ml_dtypes

import concourse.bass as bass
import concourse.mybir as mybir
import concourse.tile as tile
from concourse import bacc

F32 = mybir.dt.float32
BF16 = mybir.dt.bfloat16
AF = mybir.ActivationFunctionType
AX = mybir.AxisListType
ALU = mybir.AluOpType

N_CORES = 8


def build_nc(B_loc=2, S=4096, C=256, G=8, EPS=1e-3):
    """Build the single-core Bass program (SPMD: same program all cores)."""
    nc = bacc.Bacc(None, target_bir_lowering=False, debug=False)

    CK = C // 128          # channel chunks (2)
    NT = S // 128          # key tiles (32)
    SB = 512               # query block size
    NSB = S // SB          # query blocks (8)
    GPC = 128 // (C // G)  # groups per channel chunk (4)
    att_scale = float(C) ** -0.5
    inv_n = 1.0 / float(S * (C // G))

    x_d = nc.dram_tensor("x", [B_loc, S, C], F32, kind="ExternalInput")
    y_d = nc.dram_tensor("y", [B_loc, S, C], F32, kind="ExternalOutput")
    wq_d = nc.dram_tensor("wq", [CK, 128, C], BF16, kind="ExternalInput")
    wk_d = nc.dram_tensor("wk", [CK, 128, C], BF16, kind="ExternalInput")
    wv_d = nc.dram_tensor("wv", [CK, 128, C], BF16, kind="ExternalInput")
    wp_d = nc.dram_tensor("wp", [CK, 128, C], BF16, kind="ExternalInput")
    bq_d = nc.dram_tensor("bq", [CK, 128, 1], F32, kind="ExternalInput")
    bk_d = nc.dram_tensor("bk", [CK, 128, 1], F32, kind="ExternalInput")
    bv_d = nc.dram_tensor("bv", [1, C], F32, kind="ExternalInput")
    bp_d = nc.dram_tensor("bp", [1, C], F32, kind="ExternalInput")
    gamma_d = nc.dram_tensor("gamma", [CK, 128, 1], F32, kind="ExternalInput")
    beta_d = nc.dram_tensor("beta", [CK, 128, 1], F32, kind="ExternalInput")
    ident_d = nc.dram_tensor("ident", [128, 128], F32, kind="ExternalInput")
    onescol_d = nc.dram_tensor("onescol", [128, 1], BF16, kind="ExternalInput")
    onesrow_d = nc.dram_tensor("onesrow", [1, 128], F32, kind="ExternalInput")
    one_d = nc.dram_tensor("one", [1, 1], F32, kind="ExternalInput")
    ind_d = nc.dram_tensor("ind", [CK, 128, G], F32, kind="ExternalInput")
    indt_d = nc.dram_tensor("indt", [CK, G, 128], F32, kind="ExternalInput")

    with tile.TileContext(nc) as tc:
        with (
            tc.tile_pool(name="sb", bufs=1) as sb,
            tc.tile_pool(name="pm", bufs=1, space="PSUM") as pm,
        ):
            # ---- load constants (resident for the whole kernel) ----
            def const_tile(shape, dtype, tag):
                return sb.tile(shape, dtype, tag=tag, bufs=1, name=tag)

            wq_sb = const_tile([128, CK, C], BF16, "wq")
            wk_sb = const_tile([128, CK, C], BF16, "wk")
            wv_sb = const_tile([128, CK, C], BF16, "wv")
            wp_sb = const_tile([128, CK, C], BF16, "wp")
            for w_sb, w_d in ((wq_sb, wq_d), (wk_sb, wk_d), (wv_sb, wv_d),
                              (wp_sb, wp_d)):
                for ck in range(CK):
                    nc.sync.dma_start(w_sb[:, ck, :], w_d[ck])
            bq_sb = const_tile([128, CK], F32, "bq")
            bk_sb = const_tile([128, CK], F32, "bk")
            gamma_sb = const_tile([128, CK], F32, "gamma")
            beta_sb = const_tile([128, CK], F32, "beta")
            for b_sb, b_d in ((bq_sb, bq_d), (bk_sb, bk_d),
                              (gamma_sb, gamma_d), (beta_sb, beta_d)):
                for ck in range(CK):
                    nc.sync.dma_start(b_sb[:, ck:ck + 1], b_d[ck])
            bv_sb = const_tile([1, C], F32, "bv")
            bp_sb = const_tile([1, C], F32, "bp")
            nc.sync.dma_start(bv_sb[:], bv_d[:])
            nc.sync.dma_start(bp_sb[:], bp_d[:])
            ident_sb = const_tile([128, 128], F32, "ident")
            nc.sync.dma_start(ident_sb[:], ident_d[:])
            onescol_sb = const_tile([128, 1], BF16, "onescol")
            nc.sync.dma_start(onescol_sb[:], onescol_d[:])
            onesrow_sb = const_tile([1, 128], F32, "onesrow")
            nc.sync.dma_start(onesrow_sb[:], onesrow_d[:])
            one_sb = const_tile([1, 1], F32, "one")
            nc.sync.dma_start(one_sb[:], one_d[:])
            ind_sb = const_tile([128, CK, G], F32, "ind")
            indt_sb = const_tile([G, CK, 128], F32, "indt")
            for ck in range(CK):
                nc.sync.dma_start(ind_sb[:, ck, :], ind_d[ck])
                nc.sync.dma_start(indt_sb[:, ck, :], indt_d[ck])

            for e in range(B_loc):
                # ---- P0: load x, transpose to x_T, square-partials ----
                x_T = sb.tile([128, CK, S], F32, tag="xT", bufs=1)
                sqp = sb.tile([128, CK, NT], F32, tag="sqp", bufs=2)
                for st in range(NT):
                    stage = sb.tile([128, C], F32, tag="xs", bufs=3)
                    nc.sync.dma_start(stage[:], x_d[e, st * 128:(st + 1) * 128, :])
                    for ck in range(CK):
                        tp = pm.tile([128, 128], F32, tag="ps", bufs=3)
                        nc.tensor.matmul(tp[:], stage[:, ck * 128:(ck + 1) * 128],
                                         ident_sb[:], is_transpose=True,
                                         start=True, stop=True)
                        nc.vector.tensor_copy(x_T[:, ck, st * 128:(st + 1) * 128],
                                              tp[:])
                        trash = sb.tile([128, 128], F32, tag="trash", bufs=2)
                        nc.scalar.activation(trash[:], tp[:], AF.Square,
                                             accum_out=sqp[:, ck, st:st + 1])

                # ---- P1: group-norm statistics ----
                st2s = []
                for ck in range(CK):
                    s2 = sb.tile([128, 2], F32, tag="st2", bufs=4)
                    nc.vector.reduce_sum(s2[:, 0:1], x_T[:, ck, :], axis=AX.X)
                    nc.vector.reduce_sum(s2[:, 1:2], sqp[:, ck, :], axis=AX.X)
                    st2s.append(s2)
                gp = pm.tile([G, 2], F32, tag="ps", bufs=3)
                for ck in range(CK):
                    nc.tensor.matmul(gp[:], ind_sb[:, ck, :], st2s[ck][:],
                                     start=(ck == 0), stop=(ck == CK - 1))
                m_e = sb.tile([G, 2], F32, tag="ge", bufs=4)
                nc.scalar.mul(m_e[:], gp[:], inv_n)
                mean2 = sb.tile([G, 1], F32, tag="ge1", bufs=6)
                nc.vector.tensor_mul(mean2[:], m_e[:, 0:1], m_e[:, 0:1])
                var = sb.tile([G, 1], F32, tag="ge1", bufs=6)
                nc.vector.tensor_sub(var[:], m_e[:, 1:2], mean2[:])
                nc.vector.tensor_scalar_add(var[:], var[:], EPS)
                std = sb.tile([G, 1], F32, tag="ge1", bufs=6)
                nc.scalar.activation(std[:], var[:], AF.Sqrt)
                mr = sb.tile([G, 2], F32, tag="ge", bufs=4)
                nc.vector.tensor_copy(mr[:, 0:1], m_e[:, 0:1])
                nc.vector.reciprocal(mr[:, 1:2], std[:])

                # ---- P2: per-channel scale/bias; normalize x_T -> xn ----
                xn_f = sb.tile([128, CK, S], F32, tag="xnf", bufs=1)
                xn_b = sb.tile([128, CK, S], BF16, tag="xnb", bufs=1)
                for ck in range(CK):
                    mrc_ps = pm.tile([128, 2], F32, tag="ps", bufs=3)
                    nc.tensor.matmul(mrc_ps[:], indt_sb[:, ck, :], mr[:],
                                     start=True, stop=True)
                    mrc = sb.tile([128, 2], F32, tag="st2", bufs=4)
                    nc.vector.tensor_copy(mrc[:], mrc_ps[:])
                    scale_c = sb.tile([128, 1], F32, tag="sc", bufs=8)
                    nc.vector.tensor_mul(scale_c[:], mrc[:, 1:2],
                                         gamma_sb[:, ck:ck + 1])
                    t1 = sb.tile([128, 1], F32, tag="sc", bufs=8)
                    nc.vector.tensor_mul(t1[:], mrc[:, 0:1], scale_c[:])
                    nb = sb.tile([128, 1], F32, tag="sc", bufs=8)
                    nc.vector.tensor_sub(nb[:], beta_sb[:, ck:ck + 1], t1[:])
                    nc.scalar.activation(xn_f[:, ck, :], x_T[:, ck, :],
                                         AF.Identity, bias=nb[:], scale=scale_c[:])
                    nc.scalar.activation(xn_b[:, ck, :], x_T[:, ck, :],
                                         AF.Identity, bias=nb[:], scale=scale_c[:])

                # ---- P3: q_T, k_T (channel-major) and v (position-major) ----
                q_T = sb.tile([128, CK, S], BF16, tag="qT", bufs=1)
                k_T = sb.tile([128, CK, S], BF16, tag="kT", bufs=1)
                for w_sb, b_sb, out_t in ((wq_sb, bq_sb, q_T), (wk_sb, bk_sb, k_T)):
                    for ct in range(CK):
                        for sbk in range(NSB):
                            ps = pm.tile([128, SB], F32, tag="pb", bufs=2)
                            for kc in range(CK):
                                nc.tensor.matmul(
                                    ps[:],
                                    w_sb[:, kc, ct * 128:(ct + 1) * 128],
                                    xn_b[:, kc, sbk * SB:(sbk + 1) * SB],
                                    start=(kc == 0), stop=(kc == CK - 1))
                            nc.scalar.activation(
                                out_t[:, ct, sbk * SB:(sbk + 1) * SB], ps[:],
                                AF.Identity, bias=b_sb[:, ct:ct + 1])
                v_sb = sb.tile([128, NT, C], BF16, tag="v", bufs=1)
                for tt in range(NT):
                    ps = pm.tile([128, C], F32, tag="pb", bufs=2)
                    for kc in range(CK):
                        nc.tensor.matmul(ps[:],
                                         xn_b[:, kc, tt * 128:(tt + 1) * 128],
                                         wv_sb[:, kc, :],
                                         start=(kc == 0), stop=False)
                    nc.tensor.matmul(ps[:], onesrow_sb[:], bv_sb[:],
                                     start=False, stop=True)
                    nc.vector.tensor_copy(v_sb[:, tt, :], ps[:])

                # ---- P4: attention (scores_T -> exp -> denom + attn@v) ----
                recip_sb = sb.tile([128, NT], F32, tag="recip", bufs=2)
                for sbk in range(NSB):
                    scol = slice(sbk * SB, (sbk + 1) * SB)
                    exp_sb = sb.tile([128, NT, SB], BF16, tag="exp", bufs=1)
                    den_ps = pm.tile([1, SB], F32, tag="den", bufs=1)
                    oU0 = pm.tile([128, SB], F32, tag="accA", bufs=1)
                    oU1 = pm.tile([128, SB], F32, tag="accB", bufs=1)

                    def consume(j):
                        nc.tensor.matmul(den_ps[:], onescol_sb[:],
                                         exp_sb[:, j, :],
                                         start=(j == 0), stop=(j == NT - 1))
                        for ck, oU in ((0, oU0), (1, oU1)):
                            nc.tensor.matmul(
                                oU[:], v_sb[:, j, ck * 128:(ck + 1) * 128],
                                exp_sb[:, j, :],
                                start=(j == 0), stop=(j == NT - 1))

                    for tt in range(NT):
                        ps_s = pm.tile([128, SB], F32, tag="pb", bufs=2)
                        for ck in range(CK):
                            nc.tensor.matmul(ps_s[:],
                                             k_T[:, ck, tt * 128:(tt + 1) * 128],
                                             q_T[:, ck, scol],
                                             start=(ck == 0), stop=(ck == CK - 1))
                        nc.scalar.activation(exp_sb[:, tt, :], ps_s[:], AF.Exp,
                                             scale=att_scale)
                        if tt > 0:
                            consume(tt - 1)
                    consume(NT - 1)

                    # denominator -> per-query reciprocal in [s] layout
                    den_sb = sb.tile([1, SB], F32, tag="denc", bufs=2)
                    nc.vector.tensor_copy(den_sb[:], den_ps[:])
                    dT_ps = pm.tile([128, SB // 128], F32, tag="ps", bufs=3)
                    for j in range(SB // 128):
                        nc.tensor.matmul(dT_ps[:, j:j + 1],
                                         den_sb[0:1, j * 128:(j + 1) * 128],
                                         one_sb[:],
                                         start=(j == 0), stop=(j == SB // 128 - 1))
                    nc.vector.reciprocal(
                        recip_sb[:, sbk * (SB // 128):(sbk + 1) * (SB // 128)],
                        dT_ps[:])

                    oU_sb = sb.tile([128, CK, SB], BF16, tag="oU", bufs=2)
                    nc.vector.tensor_copy(oU_sb[:, 0, :], oU0[:])
                    nc.vector.tensor_copy(oU_sb[:, 1, :], oU1[:])

                    # ---- P5: projection + residual + output ----
                    for st in range(SB // 128):
                        gst = sbk * (SB // 128) + st
                        prj = pm.tile([128, C], F32, tag="ps", bufs=3)
                        for ck in range(CK):
                            nc.tensor.matmul(prj[:],
                                             oU_sb[:, ck, st * 128:(st + 1) * 128],
                                             wp_sb[:, ck, :],
                                             start=(ck == 0), stop=(ck == CK - 1))
                        res = pm.tile([128, C], F32, tag="ps", bufs=3)
                        for ck in range(CK):
                            nc.tensor.matmul(
                                res[:, ck * 128:(ck + 1) * 128],
                                xn_f[:, ck, gst * 128:(gst + 1) * 128],
                                ident_sb[:], is_transpose=True,
                                start=(ck == 0), stop=False)
                        nc.tensor.matmul(res[:], onesrow_sb[:], bp_sb[:],
                                         start=False, stop=True)
                        out_sb = sb.tile([128, C], F32, tag="out", bufs=3)
                        nc.vector.tensor_scalar(out_sb[:], prj[:],
                                                recip_sb[:, gst:gst + 1], None,
                                                op0=ALU.mult)
                        nc.vector.tensor_add(out_sb[:], out_sb[:], res[:])
                        nc.sync.dma_start(
                            y_d[e, gst * 128:(gst + 1) * 128, :], out_sb[:])

    return nc


def make_const_inputs(C=256, G=8):
    """Host-side constant arrays shared by all cores."""
    CK = C // 128
    cpg = C // G            # channels per group (32)
    gpc = 128 // cpg        # groups per chunk (4)
    ind = np.zeros((CK, 128, G), np.float32)
    indt = np.zeros((CK, G, 128), np.float32)
    for ck in range(CK):
        for p in range(128):
            g = ck * gpc + p // cpg
            ind[ck, p, g] = 1.0
            indt[ck, g, p] = 1.0
    return {
        "ident": np.eye(128, dtype=np.float32),
        "onescol": np.ones((128, 1), ml_dtypes.bfloat16),
        "onesrow": np.ones((1, 128), np.float32),
        "one": np.ones((1, 1), np.float32),
        "ind": ind,
        "indt": indt,
    }


def make_weight_inputs(Wq, bq, Wk, bk, Wv, bv, Wp, bp, gamma, beta):
    C = Wq.shape[0]
    CK = C // 128

    def wchunk(w):
        return np.ascontiguousarray(
            np.asarray(w, np.float32).reshape(CK, 128, C)).astype(
                ml_dtypes.bfloat16)

    def pcol(v):
        return np.ascontiguousarray(
            np.asarray(v, np.float32).reshape(CK, 128, 1))

    def row(v):
        return np.ascontiguousarray(np.asarray(v, np.float32).reshape(1, C))

    return {
        "wq": wchunk(Wq), "wk": wchunk(Wk), "wv": wchunk(Wv), "wp": wchunk(Wp),
        "bq": pcol(bq), "bk": pcol(bk), "bv": row(bv), "bp": row(bp),
        "gamma": pcol(gamma), "beta": pcol(beta),
    }


_NC_CACHE = {}


def _get_compiled_nc(B_loc, S, C):
    key = (B_loc, S, C)
    if key not in _NC_CACHE:
        nc = build_nc(B_loc=B_loc, S=S, C=C)
        nc.finalize()
        _NC_CACHE[key] = nc
    return _NC_CACHE[key]


def kernel(x, gamma, beta, Wq, bq, Wk, bk, Wv, bv, Wp, bp):
    from concourse.bass_utils import run_bass_kernel_spmd

    x = np.asarray(x, np.float32)
    B, H, W, C = x.shape
    S = H * W
    assert B % N_CORES == 0
    B_loc = B // N_CORES

    nc = _get_compiled_nc(B_loc, S, C)
    shared = make_const_inputs(C=C)
    shared.update(make_weight_inputs(Wq, bq, Wk, bk, Wv, bv, Wp, bp,
                                     gamma, beta))
    xr = x.reshape(B, S, C)
    in_maps = [
        {**shared, "x": np.ascontiguousarray(xr[k * B_loc:(k + 1) * B_loc])}
        for k in range(N_CORES)
    ]
    res = run_bass_kernel_spmd(nc, in_maps, list(range(N_CORES)))
    y = np.concatenate([res.results[k]["y"] for k in range(N_CORES)], axis=0)
    return np.ascontiguousarray(y.reshape(B, H, W, C).astype(np.float32))


# revision 33
# speedup vs baseline: 1.8886x; 1.8886x over previous
"""Trainium2 Bass kernel for an AttentionBlock (GroupNorm + single-head
self-attention + projection + residual), data-parallel over batch on 8
NeuronCores.

Reference computation (per batch element, S = H*W = 4096, C = 256):
    xn   = GroupNorm(x, groups=8, eps=1e-3) * gamma + beta
    q    = xn @ Wq + bq ; k = xn @ Wk + bk ; v = xn @ Wv + bv
    attn = softmax((q @ k^T) / sqrt(C))
    out  = attn @ v
    y    = xn + (out @ Wp + bp)

Layout strategy (per core, 2 batch elements, processed sequentially):
  - x is loaded in natural [s, c] tiles and PE-transposed to x_T [c, s].
  - GroupNorm stats come from free-dim reductions on x_T plus two tiny
    cross-partition matmuls with group indicator matrices.
  - Normalization is applied per-channel (partition scalars) on x_T,
    producing xn_T in fp32 (for the residual) and bf16 (matmul input).
  - q_T/k_T are produced in [c, s] layout, v in [t, c] layout, so the
    score matmul produces scores_T in [t, s] layout and attn @ v needs
    NO transposes of the attention matrix.
  - softmax skips the max-subtraction (scores are ~N(0,1); exp is safe)
    so everything stays in [t, s] layout; the denominator is computed
    with a ones-vector matmul and division is deferred to the very end,
    where it is a per-partition scalar in [s, c] layout.
  - The residual add reuses the PE transpose of xn_T back to [s, c].
"""

import os
import sys

for _p in ("/opt/trn_rl_repo", "/root/.axon_site/_ro/trn_rl_repo"):
    if os.path.isdir(_p) and _p not in sys.path:
        sys.path.append(_p)

import numpy as np
import ml_dtypes

import concourse.bass as bass
import concourse.mybir as mybir
import concourse.tile as tile
from concourse import bacc

F32 = mybir.dt.float32
BF16 = mybir.dt.bfloat16
AF = mybir.ActivationFunctionType
AX = mybir.AxisListType
ALU = mybir.AluOpType

N_CORES = 8


def build_nc(B_loc=2, S=4096, C=256, G=8, EPS=1e-3, f32_resid=False,
             exp_bufs=2, use_bv=True, use_bp=True):
    """Build the single-core Bass program (SPMD: same program all cores)."""
    nc = bacc.Bacc(None, target_bir_lowering=False, debug=False)

    CK = C // 128          # channel chunks (2)
    NT = S // 128          # key tiles (32)
    SB = 512               # query block size
    NSB = S // SB          # query blocks (8)
    GPC = 128 // (C // G)  # groups per channel chunk (4)
    att_scale = float(C) ** -0.5
    inv_n = 1.0 / float(S * (C // G))

    x_d = nc.dram_tensor("x", [B_loc, S, C], F32, kind="ExternalInput")
    y_d = nc.dram_tensor("y", [B_loc, S, C], F32, kind="ExternalOutput")
    wq_d = nc.dram_tensor("wq", [CK, 128, C], BF16, kind="ExternalInput")
    wk_d = nc.dram_tensor("wk", [CK, 128, C], BF16, kind="ExternalInput")
    wv_d = nc.dram_tensor("wv", [CK, 128, C], BF16, kind="ExternalInput")
    wp_d = nc.dram_tensor("wp", [CK, 128, C], BF16, kind="ExternalInput")
    bq_d = nc.dram_tensor("bq", [CK, 128, 1], F32, kind="ExternalInput")
    bk_d = nc.dram_tensor("bk", [CK, 128, 1], F32, kind="ExternalInput")
    bv_d = nc.dram_tensor("bv", [1, C], F32, kind="ExternalInput")
    bp_d = nc.dram_tensor("bp", [1, C], F32, kind="ExternalInput")
    gamma_d = nc.dram_tensor("gamma", [CK, 128, 1], F32, kind="ExternalInput")
    beta_d = nc.dram_tensor("beta", [CK, 128, 1], F32, kind="ExternalInput")
    ident_d = nc.dram_tensor("ident", [128, 128], F32, kind="ExternalInput")
    identb_d = nc.dram_tensor("identb", [128, 128], BF16, kind="ExternalInput")
    onescol_d = nc.dram_tensor("onescol", [128, 1], F32, kind="ExternalInput")
    onesrow_d = nc.dram_tensor("onesrow", [1, 128], F32, kind="ExternalInput")
    one_d = nc.dram_tensor("one", [1, 1], F32, kind="ExternalInput")
    ind_d = nc.dram_tensor("ind", [CK, 128, G], F32, kind="ExternalInput")
    indt_d = nc.dram_tensor("indt", [CK, G, 128], F32, kind="ExternalInput")

    with tile.TileContext(nc) as tc:
        with (
            tc.tile_pool(name="sb", bufs=1) as sb,
            tc.tile_pool(name="pm", bufs=1, space="PSUM") as pm,
        ):
            # ---- load constants (resident for the whole kernel) ----
            def const_tile(shape, dtype, tag):
                return sb.tile(shape, dtype, tag=tag, bufs=1, name=tag)

            wq_sb = const_tile([128, CK, C], BF16, "wq")
            wk_sb = const_tile([128, CK, C], BF16, "wk")
            wv_sb = const_tile([128, CK, C], BF16, "wv")
            wp_sb = const_tile([128, CK, C], BF16, "wp")
            for w_sb, w_d in ((wq_sb, wq_d), (wk_sb, wk_d), (wv_sb, wv_d),
                              (wp_sb, wp_d)):
                for ck in range(CK):
                    nc.gpsimd.dma_start(w_sb[:, ck, :], w_d[ck])
            bq_sb = const_tile([128, CK], F32, "bq")
            bk_sb = const_tile([128, CK], F32, "bk")
            gamma_sb = const_tile([128, CK], F32, "gamma")
            beta_sb = const_tile([128, CK], F32, "beta")
            for b_sb, b_d in ((bq_sb, bq_d), (bk_sb, bk_d),
                              (gamma_sb, gamma_d), (beta_sb, beta_d)):
                for ck in range(CK):
                    nc.gpsimd.dma_start(b_sb[:, ck:ck + 1], b_d[ck])
            bv_sb = const_tile([1, C], F32, "bv")
            bp_sb = const_tile([1, C], F32, "bp")
            nc.gpsimd.dma_start(bv_sb[:], bv_d[:])
            nc.gpsimd.dma_start(bp_sb[:], bp_d[:])
            ident_sb = const_tile([128, 128], F32, "ident")
            nc.gpsimd.dma_start(ident_sb[:], ident_d[:])
            identb_sb = const_tile([128, 128], BF16, "identb")
            nc.gpsimd.dma_start(identb_sb[:], identb_d[:])
            onescol_sb = const_tile([128, 1], F32, "onescol")
            nc.gpsimd.dma_start(onescol_sb[:], onescol_d[:])
            onesrow_sb = const_tile([1, 128], F32, "onesrow")
            nc.gpsimd.dma_start(onesrow_sb[:], onesrow_d[:])
            one_sb = const_tile([1, 1], F32, "one")
            nc.gpsimd.dma_start(one_sb[:], one_d[:])
            ind_sb = const_tile([128, CK, G], F32, "ind")
            indt_sb = const_tile([G, CK, 128], F32, "indt")
            for ck in range(CK):
                nc.gpsimd.dma_start(ind_sb[:, ck, :], ind_d[ck])
                nc.gpsimd.dma_start(indt_sb[:, ck, :], indt_d[ck])
            # bp broadcast across partitions (rank-1 matmul), for the
            # bf16-residual path where bp can't ride the residual psum
            if use_bp and not f32_resid:
                bpbc_sb = const_tile([128, C], F32, "bpbc")
                bp_ps = pm.tile([128, C], F32, tag="ps", bufs=2)
                nc.tensor.matmul(bp_ps[:], onesrow_sb[:], bp_sb[:],
                                 start=True, stop=True)
                nc.vector.tensor_copy(bpbc_sb[:], bp_ps[:])

            for e in range(B_loc):
                # ---- P0: load x, transpose to x_T, square-partials ----
                # s-tiles processed in pairs to halve evac / square op count
                x_T = sb.tile([128, CK, S], F32, tag="xT", bufs=1)
                sqp = sb.tile([128, CK, NT // 2], F32, tag="sqp", bufs=2)
                xp = sb.tile([128, CK, NT // 2], F32, tag="xp", bufs=2)
                for sp in range(NT // 2):
                    stages = []
                    for h in range(2):
                        st = 2 * sp + h
                        stage = sb.tile([128, C], F32, tag="xs", bufs=6)
                        nc.sync.dma_start(stage[:],
                                          x_d[e, st * 128:(st + 1) * 128, :])
                        stages.append(stage)
                    for ck in range(CK):
                        tp = pm.tile([128, 2, 128], F32, tag="pb", bufs=2)
                        for h in range(2):
                            nc.tensor.matmul(
                                tp[:, h, :],
                                stages[h][:, ck * 128:(ck + 1) * 128],
                                ident_sb[:], is_transpose=True,
                                start=(h == 0), stop=(h == 1))
                        nc.vector.tensor_scalar(
                            x_T[:, ck, sp * 256:(sp + 1) * 256], tp[:],
                            0.0, None, op0=ALU.add, op1=ALU.add,
                            accum_out=xp[:, ck, sp:sp + 1])
                        trash = sb.tile([128, 256], F32, tag="trash", bufs=2)
                        nc.scalar.activation(trash[:], tp[:], AF.Square,
                                             accum_out=sqp[:, ck, sp:sp + 1])

                # ---- P1: group-norm statistics ----
                st2s = []
                for ck in range(CK):
                    s2 = sb.tile([128, 2], F32, tag="st2", bufs=4)
                    nc.vector.reduce_sum(s2[:, 0:1], xp[:, ck, :], axis=AX.X)
                    nc.vector.reduce_sum(s2[:, 1:2], sqp[:, ck, :], axis=AX.X)
                    st2s.append(s2)
                gp = pm.tile([G, 2], F32, tag="ps", bufs=2)
                for ck in range(CK):
                    nc.tensor.matmul(gp[:], ind_sb[:, ck, :], st2s[ck][:],
                                     start=(ck == 0), stop=(ck == CK - 1))
                m_e = sb.tile([G, 2], F32, tag="ge", bufs=4)
                nc.scalar.mul(m_e[:], gp[:], inv_n)
                mean2 = sb.tile([G, 1], F32, tag="ge1", bufs=6)
                nc.vector.tensor_mul(mean2[:], m_e[:, 0:1], m_e[:, 0:1])
                var = sb.tile([G, 1], F32, tag="ge1", bufs=6)
                nc.vector.tensor_sub(var[:], m_e[:, 1:2], mean2[:])
                nc.vector.tensor_scalar_add(var[:], var[:], EPS)
                std = sb.tile([G, 1], F32, tag="ge1", bufs=6)
                nc.scalar.activation(std[:], var[:], AF.Sqrt)
                mr = sb.tile([G, 2], F32, tag="ge", bufs=4)
                nc.vector.tensor_copy(mr[:, 0:1], m_e[:, 0:1])
                nc.vector.reciprocal(mr[:, 1:2], std[:])

                # ---- P2: per-channel scale/bias; normalize x_T -> xn ----
                if f32_resid:
                    xn_f = sb.tile([128, CK, S], F32, tag="xnf", bufs=1)
                xn_b = sb.tile([128, CK, S], BF16, tag="xnb", bufs=1)
                for ck in range(CK):
                    mrc_ps = pm.tile([128, 2], F32, tag="ps", bufs=2)
                    nc.tensor.matmul(mrc_ps[:], indt_sb[:, ck, :], mr[:],
                                     start=True, stop=True)
                    mrc = sb.tile([128, 2], F32, tag="st2", bufs=4)
                    nc.vector.tensor_copy(mrc[:], mrc_ps[:])
                    scale_c = sb.tile([128, 1], F32, tag="sc", bufs=8)
                    nc.vector.tensor_mul(scale_c[:], mrc[:, 1:2],
                                         gamma_sb[:, ck:ck + 1])
                    t1 = sb.tile([128, 1], F32, tag="sc", bufs=8)
                    nc.vector.tensor_mul(t1[:], mrc[:, 0:1], scale_c[:])
                    nb = sb.tile([128, 1], F32, tag="sc", bufs=8)
                    nc.vector.tensor_sub(nb[:], beta_sb[:, ck:ck + 1], t1[:])
                    if f32_resid:
                        nc.scalar.activation(xn_f[:, ck, :], x_T[:, ck, :],
                                             AF.Identity, bias=nb[:],
                                             scale=scale_c[:])
                    nc.scalar.activation(xn_b[:, ck, :], x_T[:, ck, :],
                                         AF.Identity, bias=nb[:], scale=scale_c[:])
                xn_r = xn_f if f32_resid else xn_b

                # ---- P3: q_T, k_T (channel-major) and v (position-major) ----
                q_T = sb.tile([128, CK, S], BF16, tag="qT", bufs=1)
                k_T = sb.tile([128, CK, S], BF16, tag="kT", bufs=1)
                sb_pairs = [list(range(i, min(i + 2, NSB)))
                            for i in range(0, NSB, 2)]
                for w_sb, b_sb, out_t in ((wq_sb, bq_sb, q_T), (wk_sb, bk_sb, k_T)):
                    for ct in range(CK):
                        for pair in sb_pairs:
                            ps = pm.tile([128, 2, SB], F32, tag="pb", bufs=2)
                            for h, sbk in enumerate(pair):
                                for kc in range(CK):
                                    nc.tensor.matmul(
                                        ps[:, h, :],
                                        w_sb[:, kc, ct * 128:(ct + 1) * 128],
                                        xn_b[:, kc, sbk * SB:(sbk + 1) * SB],
                                        start=(kc == 0), stop=(kc == CK - 1))
                            lo, n = pair[0] * SB, len(pair) * SB
                            nc.scalar.activation(
                                out_t[:, ct, lo:lo + n], ps[:, 0:len(pair), :],
                                AF.Identity, bias=b_sb[:, ct:ct + 1])
                # v tiles in pairs: one psum bank, one accumulation group
                # (disjoint halves + pending-zero), one evac copy
                v_sb = sb.tile([128, NT, C], BF16, tag="v", bufs=1)
                for tv in range(NT // 2):
                    ps = pm.tile([128, 2, C], F32, tag="pb", bufs=2)
                    n_mm = 2 * CK + (2 if use_bv else 0)
                    i_mm = 0
                    for h in range(2):
                        tt = 2 * tv + h
                        for kc in range(CK):
                            nc.tensor.matmul(
                                ps[:, h, :],
                                xn_b[:, kc, tt * 128:(tt + 1) * 128],
                                wv_sb[:, kc, :],
                                start=(i_mm == 0), stop=(i_mm == n_mm - 1))
                            i_mm += 1
                    if use_bv:
                        for h in range(2):
                            nc.tensor.matmul(ps[:, h, :], onesrow_sb[:],
                                             bv_sb[:],
                                             start=False, stop=(i_mm == n_mm - 1))
                            i_mm += 1
                    nc.vector.tensor_copy(v_sb[:, 2 * tv:2 * tv + 2, :], ps[:])

                # ---- P4: attention (scores_T -> exp -> denom + attn@v) ----
                recip_sb = sb.tile([128, NT], F32, tag="recip", bufs=2)
                for sbk in range(NSB):
                    scol = slice(sbk * SB, (sbk + 1) * SB)
                    exp_sb = sb.tile([128, NT, SB], BF16, tag="exp",
                                     bufs=exp_bufs)
                    oU0 = pm.tile([128, SB], F32, tag="accA", bufs=1)
                    oU1 = pm.tile([128, SB], F32, tag="accB", bufs=1)

                    def consume(j):
                        for ck, oU in ((0, oU0), (1, oU1)):
                            nc.tensor.matmul(
                                oU[:], v_sb[:, j, ck * 128:(ck + 1) * 128],
                                exp_sb[:, j, :],
                                start=(j == 0), stop=(j == NT - 1))

                    # denominator partials: chunked strided reduces on DVE,
                    # emitted mid-loop so they overlap the score matmuls
                    CH = 8
                    partials = []

                    def reduce_chunk(t_hi):
                        t_lo = (len(partials)) * CH
                        p = sb.tile([128, SB], F32, tag="dpart", bufs=4)
                        nc.vector.reduce_sum(
                            p[:],
                            exp_sb[:, t_lo:t_hi, :].rearrange("p t s -> p s t"),
                            axis=AX.X)
                        partials.append(p)

                    # key tiles in pairs: 4 score matmuls, then ONE exp over
                    # both psum banks
                    for tp_i in range(NT // 2):
                        ps_s = pm.tile([128, 2, SB], F32, tag="pb", bufs=2)
                        for h in range(2):
                            tt = 2 * tp_i + h
                            for ck in range(CK):
                                nc.tensor.matmul(
                                    ps_s[:, h, :],
                                    k_T[:, ck, tt * 128:(tt + 1) * 128],
                                    q_T[:, ck, scol],
                                    start=(ck == 0), stop=(ck == CK - 1))
                        nc.scalar.activation(exp_sb[:, 2 * tp_i:2 * tp_i + 2, :],
                                             ps_s[:], AF.Exp, scale=att_scale)
                        if (2 * tp_i + 2) % CH == 0 or tp_i == NT // 2 - 1:
                            reduce_chunk(2 * tp_i + 2)
                        if tp_i > 1:
                            consume(2 * tp_i - 4)
                            consume(2 * tp_i - 3)
                    for j in range(max(0, NT - 4), NT):
                        consume(j)

                    # combine partials (tree) and one fp32 ones-matmul for the
                    # cross-partition sum
                    while len(partials) > 1:
                        nxt = []
                        for i in range(0, len(partials) - 1, 2):
                            nc.vector.tensor_add(partials[i][:], partials[i][:],
                                                 partials[i + 1][:])
                            nxt.append(partials[i])
                        if len(partials) % 2:
                            nxt.append(partials[-1])
                        partials = nxt
                    den_ps = pm.tile([1, SB], F32, tag="ps", bufs=2)
                    nc.tensor.matmul(den_ps[:], onescol_sb[:], partials[0][:],
                                     start=True, stop=True)
                    den_sb = sb.tile([1, SB], F32, tag="denc", bufs=2)
                    nc.vector.tensor_copy(den_sb[:], den_ps[:])
                    dT_ps = pm.tile([128, SB // 128], F32, tag="ps", bufs=2)
                    for j in range(SB // 128):
                        nc.tensor.matmul(dT_ps[:, j:j + 1],
                                         den_sb[0:1, j * 128:(j + 1) * 128],
                                         one_sb[:],
                                         start=(j == 0), stop=(j == SB // 128 - 1))
                    nc.vector.reciprocal(
                        recip_sb[:, sbk * (SB // 128):(sbk + 1) * (SB // 128)],
                        dT_ps[:])

                    oU_sb = sb.tile([128, CK, SB], BF16, tag="oU", bufs=2)
                    nc.vector.tensor_copy(oU_sb[:, 0, :], oU0[:])
                    nc.vector.tensor_copy(oU_sb[:, 1, :], oU1[:])

                    # ---- P5: projection + residual + output ----
                    for st in range(SB // 128):
                        gst = sbk * (SB // 128) + st
                        prj = pm.tile([128, C], F32, tag="ps", bufs=2)
                        for ck in range(CK):
                            nc.tensor.matmul(prj[:],
                                             oU_sb[:, ck, st * 128:(st + 1) * 128],
                                             wp_sb[:, ck, :],
                                             start=(ck == 0), stop=(ck == CK - 1))
                        if f32_resid:
                            res = pm.tile([128, C], F32, tag="ps", bufs=2)
                            for ck in range(CK):
                                nc.tensor.matmul(
                                    res[:, ck * 128:(ck + 1) * 128],
                                    xn_r[:, ck, gst * 128:(gst + 1) * 128],
                                    ident_sb[:], is_transpose=True,
                                    start=(ck == 0),
                                    stop=(not use_bp and ck == CK - 1))
                            if use_bp:
                                nc.tensor.matmul(res[:], onesrow_sb[:],
                                                 bp_sb[:],
                                                 start=False, stop=True)
                        else:
                            res = pm.tile([128, C], BF16, tag="ps", bufs=2)
                            for ck in range(CK):
                                nc.tensor.matmul(
                                    res[:, ck * 128:(ck + 1) * 128],
                                    xn_r[:, ck, gst * 128:(gst + 1) * 128],
                                    identb_sb[:], is_transpose=True,
                                    start=(ck == 0), stop=(ck == CK - 1))
                        out_sb = sb.tile([128, C], F32, tag="out", bufs=3)
                        nc.vector.tensor_scalar(out_sb[:], prj[:],
                                                recip_sb[:, gst:gst + 1], None,
                                                op0=ALU.mult)
                        nc.vector.tensor_add(out_sb[:], out_sb[:], res[:])
                        if use_bp and not f32_resid:
                            nc.vector.tensor_add(out_sb[:], out_sb[:],
                                                 bpbc_sb[:])
                        nc.sync.dma_start(
                            y_d[e, gst * 128:(gst + 1) * 128, :], out_sb[:])

    return nc


def make_const_inputs(C=256, G=8):
    """Host-side constant arrays shared by all cores."""
    CK = C // 128
    cpg = C // G            # channels per group (32)
    gpc = 128 // cpg        # groups per chunk (4)
    ind = np.zeros((CK, 128, G), np.float32)
    indt = np.zeros((CK, G, 128), np.float32)
    for ck in range(CK):
        for p in range(128):
            g = ck * gpc + p // cpg
            ind[ck, p, g] = 1.0
            indt[ck, g, p] = 1.0
    return {
        "ident": np.eye(128, dtype=np.float32),
        "identb": np.eye(128, dtype=np.float32).astype(ml_dtypes.bfloat16),
        "onescol": np.ones((128, 1), np.float32),
        "onesrow": np.ones((1, 128), np.float32),
        "one": np.ones((1, 1), np.float32),
        "ind": ind,
        "indt": indt,
    }


def make_weight_inputs(Wq, bq, Wk, bk, Wv, bv, Wp, bp, gamma, beta):
    C = Wq.shape[0]
    CK = C // 128

    def wchunk(w):
        return np.ascontiguousarray(
            np.asarray(w, np.float32).reshape(CK, 128, C)).astype(
                ml_dtypes.bfloat16)

    def pcol(v):
        return np.ascontiguousarray(
            np.asarray(v, np.float32).reshape(CK, 128, 1))

    def row(v):
        return np.ascontiguousarray(np.asarray(v, np.float32).reshape(1, C))

    return {
        "wq": wchunk(Wq), "wk": wchunk(Wk), "wv": wchunk(Wv), "wp": wchunk(Wp),
        "bq": pcol(bq), "bk": pcol(bk), "bv": row(bv), "bp": row(bp),
        "gamma": pcol(gamma), "beta": pcol(beta),
    }


_NC_CACHE = {}


def _get_compiled_nc(B_loc, S, C, use_bv=True, use_bp=True):
    key = (B_loc, S, C, use_bv, use_bp)
    if key not in _NC_CACHE:
        nc = build_nc(B_loc=B_loc, S=S, C=C, use_bv=use_bv, use_bp=use_bp)
        nc.finalize()
        _NC_CACHE[key] = nc
    return _NC_CACHE[key]


def kernel(x, gamma, beta, Wq, bq, Wk, bk, Wv, bv, Wp, bp):
    from concourse.bass_utils import run_bass_kernel_spmd

    x = np.asarray(x, np.float32)
    B, H, W, C = x.shape
    S = H * W
    assert B % N_CORES == 0
    B_loc = B // N_CORES

    use_bv = bool(np.any(np.asarray(bv)))
    use_bp = bool(np.any(np.asarray(bp)))
    nc = _get_compiled_nc(B_loc, S, C, use_bv, use_bp)
    shared = make_const_inputs(C=C)
    shared.update(make_weight_inputs(Wq, bq, Wk, bk, Wv, bv, Wp, bp,
                                     gamma, beta))
    xr = x.reshape(B, S, C)
    in_maps = [
        {**shared, "x": np.ascontiguousarray(xr[k * B_loc:(k + 1) * B_loc])}
        for k in range(N_CORES)
    ]
    res = run_bass_kernel_spmd(nc, in_maps, list(range(N_CORES)))
    y = np.concatenate([res.results[k]["y"] for k in range(N_CORES)], axis=0)
    return np.ascontiguousarray(y.reshape(B, H, W, C).astype(np.float32))
